# revision 1
# baseline (speedup 1.0000x reference)
"""Trainium2 Bass kernel for mutual-nearest-neighbor matching (Lowe ratio test).

Per-core layout: batch b=8 is sharded 1 batch element per NeuronCore (8 cores).
Each core computes, for its batch element:
  sim = d0^T @ d1          [n=4096, m=4096]   (bf16 matmuls, fp32 PSUM accum)
  top-2 + argmax along m  -> matches0 candidates + ratio mask + scores
  sim^T = d1^T @ d0        (second matmul direction)
  top-2 + argmax along n  -> matches1 candidates + ratio mask
  mutual check (fully local, via one small wrapped gather)
Outputs: matches int32 [4096], scores f32 [4096] per core; host stacks to [8, 4096].

Per 128-row tile (full 4096-wide row, two PSUM half-tiles):
  ACT evicts each PSUM half fp32 -> SBUF bf16 into one X row [128, 4096].
  DVE truncates the 3 low mantissa bits of the bf16 sims and folds X three
  times with pairwise max (4096 -> 512), ORing a fold-branch bit into each
  fold's right operand -- the fold winner carries its own comb-branch bits,
  so Max8(F3) + MaxIndex(F3) deliver the top-2 values AND the argmax column
  with no gather and no extra passes.  top2 is the max over everything
  outside the winner's 8-element comb group (equal to the true second max
  unless the top-2 co-locate inside one comb group -- that can only loosen
  the ratio test by 0.64*(v2-v3), far below any meaningful ratio margin).
  The 3-bit mantissa truncation perturbs sim values by <= 2^-5 ulp-of-
  exponent (~0.5% relative), well inside the bf16-level tolerance of this
  ratio test.  Ratio test + mutual check run as small batched epilogue ops;
  the mutual-check gather uses one wrapped gpsimd indirect_copy plus a
  static diagonal-mask extraction.
"""

import sys

if "/opt/trn_rl_repo" not in sys.path:
    sys.path.insert(0, "/opt/trn_rl_repo")

import numpy as np
import ml_dtypes

B, D, N, M = 8, 256, 4096, 4096
NT = N // 128            # 32 row tiles per direction
HALF = M // 2            # 2048 columns per PSUM half-tile
NBANK = HALF // 512      # 4 matmul banks per half-tile
FW = M // 8              # 512: width of the final fold array F3
NSLOT = NT               # 32 row-tile slots per direction
RATIO2 = 0.8 * 0.8       # Lowe ratio threshold squared

_CACHE: dict = {}


def _build_program(debug=False):
    import concourse.mybir as mybir
    import concourse.tile as tile
    from concourse import bacc

    dt = mybir.dt
    Alu = mybir.AluOpType

    nc = bacc.Bacc("TRN2", target_bir_lowering=False, debug=False)

    d0_dram = nc.dram_tensor("d0", [2, 128, N], dt.bfloat16, kind="ExternalInput")
    d1_dram = nc.dram_tensor("d1", [2, 128, M], dt.bfloat16, kind="ExternalInput")
    matches_dram = nc.dram_tensor("matches", [N], dt.int32, kind="ExternalOutput")
    scores_dram = nc.dram_tensor("scores", [N], dt.float32, kind="ExternalOutput")
    m1_bounce = nc.dram_tensor("m1_bounce", [M], dt.float32)  # internal
    c_indsn_dram = nc.dram_tensor("c_indsn", [128, NT], dt.float32, kind="ExternalInput")
    c_diagf_dram = nc.dram_tensor("c_diagf", [128, 16 * NT], dt.float32, kind="ExternalInput")
    if debug:
        dbg_m0 = nc.dram_tensor("dbg_m0", [N], dt.float32, kind="ExternalOutput")
        dbg_m1 = nc.dram_tensor("dbg_m1", [M], dt.float32, kind="ExternalOutput")
        dbg_loop = nc.dram_tensor("dbg_loop", [N], dt.float32, kind="ExternalOutput")
        dbg_inds = nc.dram_tensor("dbg_inds", [N], dt.float32, kind="ExternalOutput")

    with tile.TileContext(nc) as tc:
        with (
            tc.tile_pool(name="w", bufs=1) as wpool,
            tc.tile_pool(name="consts", bufs=1) as cpool,
            tc.tile_pool(name="acc", bufs=1) as apool,
            tc.tile_pool(name="x", bufs=6) as xpool,
            tc.tile_pool(name="f", bufs=4) as fpool,
            tc.tile_pool(name="psum", bufs=2, space="PSUM") as ppool,
        ):
            # ---- load descriptors (already bf16, k-major [2, 128, N]) ----
            d0_sb = [wpool.tile([128, N], dt.bfloat16, name=f"d0_{k}") for k in range(2)]
            d1_sb = [wpool.tile([128, M], dt.bfloat16, name=f"d1_{k}") for k in range(2)]
            for k in range(2):
                nc.sync.dma_start(d0_sb[k][:], d0_dram[k])
                nc.sync.dma_start(d1_sb[k][:], d1_dram[k])

            # ---- constants (host-provided) ----
            indsn = cpool.tile([128, NT], dt.float32, name="indsn")
            nc.sync.dma_start(indsn[:], c_indsn_dram[:])
            diag_f = cpool.tile([128, 16 * NT], dt.float32, name="diag_f")
            nc.sync.dma_start(diag_f[:], c_diagf_dram[:])

            # ---- per-direction accumulators ----
            t8_acc = [apool.tile([128, NSLOT * 8], dt.bfloat16, name=f"t8_{d}") for d in range(2)]
            pi_acc = [apool.tile([128, NSLOT * 8], dt.uint16, name=f"pi_{d}") for d in range(2)]

            m_dir = [apool.tile([128, NT], dt.float32, name=f"mdir_{d}") for d in range(2)]
            scores0 = apool.tile([128, NT], dt.float32, name="scores0")

            for dire in range(2):
                lhs = d0_sb if dire == 0 else d1_sb
                rhs = d1_sb if dire == 0 else d0_sb
                t8a, pia = t8_acc[dire], pi_acc[dire]

                for t in range(NT):
                    s = t
                    X = xpool.tile([128, M], dt.bfloat16, name=f"X_{dire}_{s}", tag="X")
                    for h in range(2):
                        P = ppool.tile([128, HALF], dt.float32, name=f"P_{dire}_{s}_{h}", tag="P")
                        for k in range(2):
                            for b in range(NBANK):
                                nc.tensor.matmul(
                                    P[:, 512 * b : 512 * (b + 1)],
                                    lhs[k][:, 128 * t : 128 * (t + 1)],
                                    rhs[k][:, HALF * h + 512 * b : HALF * h + 512 * (b + 1)],
                                    start=(k == 0),
                                    stop=(k == 1),
                                )
                        nc.scalar.copy(X[:, HALF * h : HALF * (h + 1)], P[:])
                    # bit-packed folds: truncate the 3 low mantissa bits and OR a
                    # fold-branch bit into each fold's right operand.  The fold
                    # winner then carries its own comb-branch bits.
                    Xu = X[:].bitcast(dt.uint16)
                    XL = fpool.tile([128, M // 2], dt.bfloat16, name=f"XL_{dire}_{s}", tag="XL")
                    nc.vector.tensor_scalar(
                        XL[:].bitcast(dt.uint16), Xu[:, : M // 2], 0xFFF8, None,
                        op0=Alu.bitwise_and,
                    )
                    XR = fpool.tile([128, M // 2], dt.bfloat16, name=f"XR_{dire}_{s}", tag="XR")
                    nc.vector.tensor_scalar(
                        XR[:].bitcast(dt.uint16), Xu[:, M // 2 :], 0xFFF8, 1,
                        op0=Alu.bitwise_and, op1=Alu.bitwise_or,
                    )
                    F1 = fpool.tile([128, M // 2], dt.bfloat16, name=f"F1_{dire}_{s}", tag="F1")
                    nc.vector.tensor_max(F1[:], XL[:], XR[:])
                    FR2 = fpool.tile([128, M // 4], dt.bfloat16, name=f"FR2_{dire}_{s}", tag="FR2")
                    nc.vector.tensor_scalar(
                        FR2[:].bitcast(dt.uint16), F1[:].bitcast(dt.uint16)[:, M // 4 :], 2, None,
                        op0=Alu.bitwise_or,
                    )
                    F2 = fpool.tile([128, M // 4], dt.bfloat16, name=f"F2_{dire}_{s}", tag="F2")
                    nc.vector.tensor_max(F2[:], F1[:, : M // 4], FR2[:])
                    FR3 = fpool.tile([128, FW], dt.bfloat16, name=f"FR3_{dire}_{s}", tag="FR3")
                    nc.vector.tensor_scalar(
                        FR3[:].bitcast(dt.uint16), F2[:].bitcast(dt.uint16)[:, FW:], 4, None,
                        op0=Alu.bitwise_or,
                    )
                    F3 = fpool.tile([128, FW], dt.bfloat16, name=f"F3_{dire}_{s}", tag="F3")
                    nc.vector.tensor_max(F3[:], F2[:, :FW], FR3[:])

                    t8_slot = t8a[:, 8 * s : 8 * s + 8]
                    pi_slot = pia[:, 8 * s : 8 * s + 8]
                    nc.vector.max(t8_slot, F3[:])
                    nc.vector.max_index(pi_slot, t8_slot, F3[:])

                # ---- batched epilogue for this direction ----
                # strip the embedded index bits from the stored top-8 values
                t8c = apool.tile([128, NSLOT * 8], dt.bfloat16, name=f"t8c_{dire}", tag="t8c")
                nc.vector.tensor_scalar(
                    t8c[:].bitcast(dt.uint16), t8a[:].bitcast(dt.uint16), 0xFFF8, None,
                    op0=Alu.bitwise_and,
                )
                A3 = t8c[:].rearrange("p (g e) -> p g e", e=8)
                A3u = t8a[:].bitcast(dt.uint16).rearrange("p (g e) -> p g e", e=8)
                P3 = pia[:].rearrange("p (g e) -> p g e", e=8)

                v1g = apool.tile([128, NT], dt.float32, name=f"v1g_{dire}", tag="v1g")
                nc.vector.tensor_copy(v1g[:], A3[:, :, 0])
                v2g = apool.tile([128, NT], dt.float32, name=f"v2g_{dire}", tag="v2g")
                nc.vector.tensor_copy(v2g[:], A3[:, :, 1])
                pf = apool.tile([128, NSLOT], dt.float32, name=f"pf_{dire}", tag="pf")
                nc.vector.tensor_copy(pf[:], P3[:, :, 0])

                # decode the winner's branch bits: bit0 (X level, weight 2048),
                # bit1 (F1 level, raw value 2 -> weight 1024), bit2 (F2 level,
                # raw value 4 -> weight 512)
                b0u = apool.tile([128, NSLOT], dt.uint16, name=f"b0u_{dire}", tag="b0u")
                nc.vector.tensor_scalar(b0u[:], A3u[:, :, 0], 1, None, op0=Alu.bitwise_and)
                b1u = apool.tile([128, NSLOT], dt.uint16, name=f"b1u_{dire}", tag="b1u")
                nc.vector.tensor_scalar(b1u[:], A3u[:, :, 0], 2, None, op0=Alu.bitwise_and)
                b2u = apool.tile([128, NSLOT], dt.uint16, name=f"b2u_{dire}", tag="b2u")
                nc.vector.tensor_scalar(b2u[:], A3u[:, :, 0], 4, None, op0=Alu.bitwise_and)
                b0f = apool.tile([128, NSLOT], dt.float32, name=f"b0f_{dire}", tag="b0f")
                nc.vector.tensor_copy(b0f[:], b0u[:])
                b1f = apool.tile([128, NSLOT], dt.float32, name=f"b1f_{dire}", tag="b1f")
                nc.vector.tensor_copy(b1f[:], b1u[:])
                b2f = apool.tile([128, NSLOT], dt.float32, name=f"b2f_{dire}", tag="b2f")
                nc.vector.tensor_copy(b2f[:], b2u[:])

                # absolute column index: m = p + 2048*b0 + 1024*(b1/2) + 512*(b2/4)
                mst = apool.tile([128, NSLOT], dt.float32, name=f"mst_{dire}", tag="mst")
                nc.vector.scalar_tensor_tensor(
                    mst[:], b0f[:], 2048.0, pf[:], op0=Alu.mult, op1=Alu.add
                )
                nc.vector.scalar_tensor_tensor(
                    mst[:], b1f[:], 512.0, mst[:], op0=Alu.mult, op1=Alu.add
                )
                nc.vector.scalar_tensor_tensor(
                    mst[:], b2f[:], 128.0, mst[:], op0=Alu.mult, op1=Alu.add
                )

                # ratio test: dist1 <= r^2 * dist2  <=>  v1 - r^2*v2 >= 1 - r^2
                acc1 = apool.tile([128, NT], dt.float32, name=f"acc1_{dire}", tag="acc1")
                nc.vector.scalar_tensor_tensor(
                    acc1[:], v2g[:], -RATIO2, v1g[:], op0=Alu.mult, op1=Alu.add
                )
                maskf = apool.tile([128, NT], dt.uint8, name=f"maskf_{dire}", tag="maskf")
                nc.vector.tensor_scalar(
                    maskf[:], acc1[:], 1.0 - RATIO2, None, op0=Alu.is_ge
                )
                if dire == 0:
                    sc = apool.tile([128, NT], dt.float32, name="sc")
                    nc.vector.tensor_scalar(
                        sc[:], v1g[:], 0.5, 0.5, op0=Alu.mult, op1=Alu.add
                    )
                    nc.vector.tensor_mul(scores0[:], sc[:], maskf[:])
                nc.vector.memset(m_dir[dire][:], -1.0)
                nc.vector.copy_predicated(m_dir[dire][:], maskf[:], mst[:])

            # ---- mutual check ----
            m1_flat_ap = m1_bounce[:].rearrange("(t r) -> r t", r=128)
            nc.sync.dma_start(m1_flat_ap, m_dir[1][:])
            m1_rep = apool.tile([128, M], dt.float32, name="m1_rep")
            nc.sync.dma_start(m1_rep[:1, :], m1_bounce[:][None, :])
            nc.gpsimd.partition_broadcast(m1_rep[:, :], m1_rep[:1, :])

            safe = apool.tile([128, NT], dt.float32, name="safe")
            nc.vector.tensor_scalar_max(safe[:], m_dir[0][:], 0.0)
            safe16 = apool.tile([128, NT], dt.uint16, name="safe16")
            nc.vector.tensor_copy(safe16[:], safe[:])
            gm = apool.tile([128, 16 * NT], dt.float32, name="gm")
            nc.gpsimd.indirect_copy(gm[:], m1_rep[:], safe16[:], True)
            gmp = apool.tile([128, 16 * NT], dt.float32, name="gmp")
            nc.vector.tensor_mul(gmp[:], gm[:], diag_f[:])
            loop = apool.tile([128, NT], dt.float32, name="loop")
            nc.vector.tensor_reduce(
                loop[:],
                gmp[:].rearrange("p (j u) -> p j u", u=16),
                axis=mybir.AxisListType.X,
                op=Alu.add,
            )

            g1 = apool.tile([128, NT], dt.uint8, name="g1")
            nc.vector.tensor_scalar(g1[:], m_dir[0][:], -0.5, None, op0=Alu.is_gt)
            g2 = apool.tile([128, NT], dt.uint8, name="g2")
            nc.vector.tensor_tensor(g2[:], indsn[:], loop[:], op=Alu.is_equal)
            okm = apool.tile([128, NT], dt.uint8, name="okm")
            nc.vector.tensor_mul(okm[:], g1[:], g2[:])

            mfin = apool.tile([128, NT], dt.float32, name="mfin")
            nc.vector.memset(mfin[:], -1.0)
            nc.vector.copy_predicated(mfin[:], okm[:], m_dir[0][:])
            mi32 = apool.tile([128, NT], dt.int32, name="mi32")
            nc.vector.tensor_copy(mi32[:], mfin[:])

            nc.sync.dma_start(matches_dram[:].rearrange("(t r) -> r t", r=128), mi32[:])
            nc.sync.dma_start(scores_dram[:].rearrange("(t r) -> r t", r=128), scores0[:])
            if debug:
                nc.sync.dma_start(dbg_m0[:].rearrange("(t r) -> r t", r=128), m_dir[0][:])
                nc.sync.dma_start(dbg_m1[:].rearrange("(t r) -> r t", r=128), m_dir[1][:])
                nc.sync.dma_start(dbg_loop[:].rearrange("(t r) -> r t", r=128), loop[:])
                nc.sync.dma_start(dbg_inds[:].rearrange("(t r) -> r t", r=128), indsn[:])

    nc.compile()
    return nc


def _get_program():
    if "nc" not in _CACHE:
        _CACHE["nc"] = _build_program()
    return _CACHE["nc"]


def _make_consts():
    if "consts" in _CACHE:
        return _CACHE["consts"]
    p = np.arange(128)
    c_indsn = (128 * np.arange(NT)[None, :] + p[:, None]).astype(np.float32)
    diag = (np.arange(16)[None, :] == (p % 16)[:, None])  # [128, 16]
    c_diagf = np.tile(diag, (1, NT)).astype(np.float32)
    consts = {"c_indsn": c_indsn, "c_diagf": c_diagf}
    _CACHE["consts"] = consts
    return consts


def _make_in_maps(descriptors0, descriptors1):
    consts = _make_consts()
    in_maps = []
    for c in range(B):
        a = np.ascontiguousarray(descriptors0[c].reshape(2, 128, N)).astype(
            ml_dtypes.bfloat16
        )
        bb = np.ascontiguousarray(descriptors1[c].reshape(2, 128, M)).astype(
            ml_dtypes.bfloat16
        )
        in_maps.append({"d0": a, "d1": bb, **consts})
    return in_maps


def kernel(descriptors0: np.ndarray, descriptors1: np.ndarray):
    from concourse.bass_utils import run_bass_kernel_spmd

    nc = _get_program()
    in_maps = _make_in_maps(descriptors0, descriptors1)
    res = run_bass_kernel_spmd(nc, in_maps, core_ids=list(range(B)))
    matches = np.stack([np.asarray(res.results[c]["matches"]) for c in range(B)])
    scores = np.stack([np.asarray(res.results[c]["scores"]) for c in range(B)])
    return matches.astype(np.int32), scores.astype(np.float32)



# revision 6
# speedup vs baseline: 1.1717x; 1.1717x over previous
"""Trainium2 Bass kernel for mutual-nearest-neighbor matching (Lowe ratio test).

Batch b=8 sharded 1 element per NeuronCore. Per core:
  sim = d0^T @ d1  [4096, 4096] via fp8-e4m3 DoubleRow matmuls (K=256 in one
  pass, descriptors host-scaled by 16 so sims live in a x256 fp32 domain).

Direction 0 (rows): per 128-row tile, ACT evicts both PSUM halves to bf16 X
  [128, 4096]; DVE folds X -> F1 -> F2 -> F3 (pure tensor_max, full bf16
  precision, no bit embedding), Max8 + FindIndex8 on F3 [512] give
  (v1, v2, F3-slot j*).  The winning column is recovered exactly by one
  gpsimd gather of the 8 comb candidates X[j* + 512k] and a batched
  arithmetic decode in the epilogue.  v2 equals the true second max unless
  the top-2 co-locate in one 8-column comb (harmless for the ratio test).

Direction 1 (columns): the mutual check only needs, per column c, the
  bf16 column max V1[c] and its ratio mask -- no argmax index: row r is the
  column argmax  iff  v1_dir0[r] == V1[c] (bf16 maxes of the same bit-exact
  bf16 sims).  So dir-1 tiles are matmul + fold + Max8 only; a fraction fold
  the second PSUM half directly (tensor_max(SBUF bf16, PSUM fp32)) which
  skips one ACT eviction per tile to balance engine load.  V1 (masked, with
  failed columns set to an impossible value) is bounced through DRAM,
  partition-broadcast, and gathered at m0 for the mutual test.

Engines: PE fp8 matmuls; ACT PSUM evictions; DVE folds/max8/find8;
  Pool (gpsimd) dir-1 F2/F3 folds, candidate gathers and extractions.
"""

import sys

if "/opt/trn_rl_repo" not in sys.path:
    sys.path.insert(0, "/opt/trn_rl_repo")

import numpy as np
import ml_dtypes

B, D, N, M = 8, 256, 4096, 4096
NT = N // 128            # 32 row tiles per direction
HALF = M // 2            # 2048 columns per PSUM half-tile
SCALE = 16.0             # host descriptor scale; sims carry SCALE^2 = 256
RATIO2 = 0.8 * 0.8
THRESH = (1.0 - RATIO2) * SCALE * SCALE   # 0.36 * 256 = 92.16
IMPOSSIBLE = 2.1 * SCALE * SCALE          # > any sim*256
# dir-1 tiles with (t % 3 != 0) evict both halves via ACT;
# the rest fold the second half straight from PSUM on DVE.

_CACHE: dict = {}


def _build_program():
    import concourse.mybir as mybir
    import concourse.tile as tile
    from concourse import bacc

    dt = mybir.dt
    Alu = mybir.AluOpType
    DR = mybir.MatmulPerfMode.DoubleRow

    nc = bacc.Bacc("TRN2", target_bir_lowering=False, debug=False)

    d0_dram = nc.dram_tensor("d0", [128, 2, N], dt.float8e4, kind="ExternalInput")
    d1_dram = nc.dram_tensor("d1", [128, 2, M], dt.float8e4, kind="ExternalInput")
    matches_dram = nc.dram_tensor("matches", [N], dt.int32, kind="ExternalOutput")
    scores_dram = nc.dram_tensor("scores", [N], dt.float32, kind="ExternalOutput")
    v1_bounce = nc.dram_tensor("v1_bounce", [M], dt.float32)  # internal
    c_off8_dram = nc.dram_tensor("c_off8", [128, 8], dt.uint16, kind="ExternalInput")
    c_diag128_dram = nc.dram_tensor("c_diag128", [128, 128], dt.bfloat16, kind="ExternalInput")
    c_diagf512_dram = nc.dram_tensor("c_diagf512", [128, 512], dt.float32, kind="ExternalInput")
    c_prio_dram = nc.dram_tensor("c_prio", [128, 128], dt.float32, kind="ExternalInput")

    with tile.TileContext(nc) as tc:
        with (
            tc.tile_pool(name="w", bufs=1) as wpool,
            tc.tile_pool(name="acc", bufs=1) as apool,
            tc.tile_pool(name="x", bufs=4) as xpool,
            tc.tile_pool(name="f", bufs=4) as fpool,
            tc.tile_pool(name="g", bufs=4) as gpool,
            tc.tile_pool(name="psum", bufs=2, space="PSUM") as ppool,
        ):
            # ---- load descriptors (fp8, k = subtile*128 + partition) ----
            d0_sb = wpool.tile([128, 2, N], dt.float8e4, name="d0")
            nc.sync.dma_start(d0_sb[:], d0_dram[:])
            d1_sb = wpool.tile([128, 2, M], dt.float8e4, name="d1")
            nc.sync.dma_start(d1_sb[:], d1_dram[:])

            # ---- constants ----
            c_off8 = wpool.tile([128, 8], dt.uint16, name="c_off8")
            nc.sync.dma_start(c_off8[:], c_off8_dram[:])
            c_diag128 = wpool.tile([128, 128], dt.bfloat16, name="c_diag128")
            nc.sync.dma_start(c_diag128[:], c_diag128_dram[:])
            c_diagf512 = wpool.tile([128, 512], dt.float32, name="c_diagf512")
            nc.sync.dma_start(c_diagf512[:], c_diagf512_dram[:])
            c_prio = wpool.tile([128, 128], dt.float32, name="c_prio")
            nc.sync.dma_start(c_prio[:], c_prio_dram[:])

            # ---- accumulators ----
            t8a0 = apool.tile([128, NT * 8], dt.bfloat16, name="t8a0")
            t8a1 = apool.tile([128, NT * 8], dt.bfloat16, name="t8a1")
            jacc = apool.tile([128, NT], dt.float32, name="jacc")
            gacc = apool.tile([128, NT * 8], dt.float32, name="gacc")
            V1R = apool.tile([128, M], dt.float32, name="V1R")

            def mm_tile(P, lhs, rhs, t, h):
                for bk in range(4):
                    nc.tensor.matmul(
                        P[:, 512 * bk : 512 * (bk + 1)],
                        lhs[:, :, 128 * t : 128 * (t + 1)],
                        rhs[:, :, HALF * h + 512 * bk : HALF * h + 512 * (bk + 1)],
                        start=True,
                        stop=True,
                        perf_mode=DR,
                    )

            for t in range(NT):
                # ================= direction 1 (columns), tile t ============
                evict = (t % 3) != 0
                P0 = ppool.tile([128, HALF], dt.float32, name=f"q0_{t}", tag="P")
                mm_tile(P0, d1_sb, d0_sb, t, 0)
                P1 = ppool.tile([128, HALF], dt.float32, name=f"q1_{t}", tag="P")
                mm_tile(P1, d1_sb, d0_sb, t, 1)
                F1q = fpool.tile([128, HALF], dt.bfloat16, name=f"f1q_{t}", tag="F1")
                Xq = xpool.tile([128, M], dt.bfloat16, name=f"xq_{t}", tag="X")
                if evict:
                    nc.scalar.copy(Xq[:, :HALF], P0[:])
                    nc.scalar.copy(Xq[:, HALF:], P1[:])
                    nc.vector.tensor_max(F1q[:], Xq[:, :HALF], Xq[:, HALF:])
                else:
                    nc.scalar.copy(Xq[:, :HALF], P0[:])
                    nc.vector.tensor_max(F1q[:], Xq[:, :HALF], P1[:])
                F2q = fpool.tile([128, 1024], dt.bfloat16, name=f"f2q_{t}", tag="F2")
                nc.vector.tensor_max(F2q[:], F1q[:, :1024], F1q[:, 1024:])
                F3q = fpool.tile([128, 512], dt.bfloat16, name=f"f3q_{t}", tag="F3")
                nc.vector.tensor_max(F3q[:], F2q[:, :512], F2q[:, 512:])
                nc.vector.max(t8a1[:, 8 * t : 8 * t + 8], F3q[:])

                # ================= direction 0 (rows), tile t ===============
                R0 = ppool.tile([128, HALF], dt.float32, name=f"r0_{t}", tag="P")
                mm_tile(R0, d0_sb, d1_sb, t, 0)
                R1 = ppool.tile([128, HALF], dt.float32, name=f"r1_{t}", tag="P")
                mm_tile(R1, d0_sb, d1_sb, t, 1)
                X = xpool.tile([128, M], dt.bfloat16, name=f"x_{t}", tag="X")
                nc.scalar.copy(X[:, :HALF], R0[:])
                nc.scalar.copy(X[:, HALF:], R1[:])
                F1 = fpool.tile([128, HALF], dt.bfloat16, name=f"f1_{t}", tag="F1")
                nc.vector.tensor_max(F1[:], X[:, :HALF], X[:, HALF:])
                F2 = fpool.tile([128, 1024], dt.bfloat16, name=f"f2_{t}", tag="F2")
                nc.vector.tensor_max(F2[:], F1[:, :1024], F1[:, 1024:])
                F3 = fpool.tile([128, 512], dt.bfloat16, name=f"f3_{t}", tag="F3")
                nc.vector.tensor_max(F3[:], F2[:, :512], F2[:, 512:])
                t8s = t8a0[:, 8 * t : 8 * t + 8]
                nc.vector.max(t8s, F3[:])
                pi = gpool.tile([128, 8], dt.uint16, name=f"pi_{t}", tag="pi")
                nc.vector.max_index(pi[:], t8s, F3[:])
                nc.vector.tensor_copy(jacc[:, t : t + 1], pi[:, 0:1])
                idx8 = gpool.tile([128, 8], dt.uint16, name=f"ix_{t}", tag="ix")
                nc.vector.tensor_scalar(
                    idx8[:], c_off8[:], jacc[:, t : t + 1], None, op0=Alu.add
                )
                gth = gpool.tile([128, 128], dt.bfloat16, name=f"gt_{t}", tag="gt")
                nc.gpsimd.indirect_copy(gth[:], X[:], idx8[:], True)
                gd = gpool.tile([128, 128], dt.float32, name=f"gd_{t}", tag="gd")
                nc.vector.tensor_mul(gd[:], gth[:], c_diag128[:])
                nc.vector.tensor_reduce(
                    gacc[:, 8 * t : 8 * t + 8],
                    gd[:].rearrange("p (j u) -> p j u", u=16),
                    axis=mybir.AxisListType.X,
                    op=Alu.add,
                )

            # ============== dir-1 epilogue: masked column maxes =============
            v1_1 = apool.tile([128, NT], dt.float32, name="v1_1")
            nc.vector.tensor_copy(v1_1[:], t8a1[:].rearrange("p (t e) -> p t e", e=8)[:, :, 0])
            v2_1 = apool.tile([128, NT], dt.float32, name="v2_1")
            nc.vector.tensor_copy(v2_1[:], t8a1[:].rearrange("p (t e) -> p t e", e=8)[:, :, 1])
            r1 = apool.tile([128, NT], dt.float32, name="r1e")
            nc.vector.scalar_tensor_tensor(
                r1[:], v2_1[:], -RATIO2, v1_1[:], op0=Alu.mult, op1=Alu.add
            )
            mask1 = apool.tile([128, NT], dt.uint8, name="mask1")
            nc.vector.tensor_scalar(mask1[:], r1[:], THRESH, None, op0=Alu.is_ge)
            V1M = apool.tile([128, NT], dt.float32, name="V1M")
            nc.vector.memset(V1M[:], IMPOSSIBLE)
            nc.vector.copy_predicated(V1M[:], mask1[:], v1_1[:])
            nc.sync.dma_start(v1_bounce[:].rearrange("(t r) -> r t", r=128), V1M[:])
            nc.sync.dma_start(V1R[:1, :], v1_bounce[:][None, :])
            nc.gpsimd.partition_broadcast(V1R[:, :], V1R[:1, :])

            # ============== dir-0 epilogue: decode + mask + mutual ==========
            v1_0 = apool.tile([128, NT], dt.float32, name="v1_0")
            nc.vector.tensor_copy(v1_0[:], t8a0[:].rearrange("p (t e) -> p t e", e=8)[:, :, 0])
            v2_0 = apool.tile([128, NT], dt.float32, name="v2_0")
            nc.vector.tensor_copy(v2_0[:], t8a0[:].rearrange("p (t e) -> p t e", e=8)[:, :, 1])

            # comb decode: gacc[:, 8t+k] = X[j*+512k] (k<4 left, k>=4 right)
            gv = gacc[:].rearrange("p (t k) -> p t k", k=8)
            XLc = gv[:, :, 0:4]
            XRc = gv[:, :, 4:8]
            F1c = apool.tile([128, NT * 4], dt.float32, name="F1c")
            F1cv = F1c[:].rearrange("p (t k) -> p t k", k=4)
            nc.vector.tensor_tensor(F1cv, XLc, XRc, op=Alu.max)
            # v1 broadcast x4
            v1x4 = apool.tile([128, NT * 4], dt.float32, name="v1x4")
            v1x4v = v1x4[:].rearrange("p (t k) -> p t k", k=4)
            for k in range(4):
                nc.vector.tensor_copy(
                    v1x4v[:, :, k : k + 1],
                    v1_0[:].rearrange("p (t o) -> p t o", o=1),
                )
            eqk = apool.tile([128, NT * 4], dt.uint8, name="eqk")
            nc.vector.tensor_tensor(eqk[:], F1c[:], v1x4[:], op=Alu.is_equal)
            sck = apool.tile([128, NT * 4], dt.float32, name="sck")
            nc.vector.tensor_mul(sck[:], eqk[:], c_prio[:])
            mo = apool.tile([128, NT], dt.float32, name="mo")
            nc.vector.tensor_reduce(
                mo[:], sck[:].rearrange("p (t k) -> p t k", k=4),
                axis=mybir.AxisListType.X, op=Alu.max,
            )
            # l_off = 2048 - mo ; onehot = (sck == mo)
            l_off = apool.tile([128, NT], dt.float32, name="l_off")
            nc.vector.tensor_scalar(l_off[:], mo[:], -1.0, 2048.0, op0=Alu.mult, op1=Alu.add)
            mox4 = apool.tile([128, NT * 4], dt.float32, name="mox4")
            mox4v = mox4[:].rearrange("p (t k) -> p t k", k=4)
            for k in range(4):
                nc.vector.tensor_copy(
                    mox4v[:, :, k : k + 1],
                    mo[:].rearrange("p (t o) -> p t o", o=1),
                )
            onehot = apool.tile([128, NT * 4], dt.uint8, name="onehot")
            nc.vector.tensor_tensor(onehot[:], sck[:], mox4[:], op=Alu.is_equal)
            XLs = apool.tile([128, NT * 4], dt.float32, name="XLs")
            nc.vector.tensor_tensor(XLs[:], onehot[:], XLc, op=Alu.mult)
            XLsel = apool.tile([128, NT], dt.float32, name="XLsel")
            nc.vector.tensor_reduce(
                XLsel[:], XLs[:].rearrange("p (t k) -> p t k", k=4),
                axis=mybir.AxisListType.X, op=Alu.add,
            )
            bitR = apool.tile([128, NT], dt.uint8, name="bitR")
            nc.vector.tensor_tensor(bitR[:], XLsel[:], v1_0[:], op=Alu.is_lt)
            # col = j* + l_off + 2048*bitR
            colf = apool.tile([128, NT], dt.float32, name="colf")
            nc.vector.tensor_tensor(colf[:], jacc[:], l_off[:], op=Alu.add)
            nc.vector.scalar_tensor_tensor(
                colf[:], bitR[:], 2048.0, colf[:], op0=Alu.mult, op1=Alu.add
            )

            # ratio mask + scores
            r0e = apool.tile([128, NT], dt.float32, name="r0e")
            nc.vector.scalar_tensor_tensor(
                r0e[:], v2_0[:], -RATIO2, v1_0[:], op0=Alu.mult, op1=Alu.add
            )
            mask0 = apool.tile([128, NT], dt.uint8, name="mask0")
            nc.vector.tensor_scalar(mask0[:], r0e[:], THRESH, None, op0=Alu.is_ge)
            sc = apool.tile([128, NT], dt.float32, name="sc")
            nc.vector.tensor_scalar(
                sc[:], v1_0[:], 0.5 / (SCALE * SCALE), 0.5, op0=Alu.mult, op1=Alu.add
            )
            scores0 = apool.tile([128, NT], dt.float32, name="scores0")
            nc.vector.tensor_mul(scores0[:], sc[:], mask0[:])

            m0 = apool.tile([128, NT], dt.float32, name="m0")
            nc.vector.memset(m0[:], -1.0)
            nc.vector.copy_predicated(m0[:], mask0[:], colf[:])

            # mutual: gather V1R at columns m0 and compare with v1_0
            safe = apool.tile([128, NT], dt.float32, name="safe")
            nc.vector.tensor_scalar_max(safe[:], m0[:], 0.0)
            safe16 = apool.tile([128, NT], dt.uint16, name="safe16")
            nc.vector.tensor_copy(safe16[:], safe[:])
            gvr = apool.tile([128, 16 * NT], dt.float32, name="gvr")
            nc.gpsimd.indirect_copy(gvr[:], V1R[:], safe16[:], True)
            gvd = apool.tile([128, 16 * NT], dt.float32, name="gvd")
            nc.vector.tensor_mul(gvd[:], gvr[:], c_diagf512[:])
            V1at = apool.tile([128, NT], dt.float32, name="V1at")
            nc.vector.tensor_reduce(
                V1at[:], gvd[:].rearrange("p (j u) -> p j u", u=16),
                axis=mybir.AxisListType.X, op=Alu.add,
            )
            okb = apool.tile([128, NT], dt.uint8, name="okb")
            nc.vector.tensor_tensor(okb[:], V1at[:], v1_0[:], op=Alu.is_equal)
            ok = apool.tile([128, NT], dt.uint8, name="ok")
            nc.vector.tensor_mul(ok[:], okb[:], mask0[:])

            mfin = apool.tile([128, NT], dt.float32, name="mfin")
            nc.vector.memset(mfin[:], -1.0)
            nc.vector.copy_predicated(mfin[:], ok[:], m0[:])
            mi32 = apool.tile([128, NT], dt.int32, name="mi32")
            nc.vector.tensor_copy(mi32[:], mfin[:])

            nc.sync.dma_start(matches_dram[:].rearrange("(t r) -> r t", r=128), mi32[:])
            nc.sync.dma_start(scores_dram[:].rearrange("(t r) -> r t", r=128), scores0[:])

    nc.compile()
    return nc


def _get_program():
    if "nc" not in _CACHE:
        _CACHE["nc"] = _build_program()
    return _CACHE["nc"]


def _make_consts():
    if "consts" in _CACHE:
        return _CACHE["consts"]
    p = np.arange(128)
    diag16 = (np.arange(16)[None, :] == (p % 16)[:, None])  # [128, 16]
    off8 = np.array([0, 512, 1024, 1536, 2048, 2560, 3072, 3584], dtype=np.uint16)
    consts = {
        "c_off8": np.tile(off8[None, :], (128, 1)).astype(np.uint16),
        "c_diag128": np.tile(diag16, (1, 8)).astype(ml_dtypes.bfloat16),
        "c_diagf512": np.tile(diag16, (1, 32)).astype(np.float32),
        "c_prio": np.tile(
            np.repeat(np.array([2048.0, 1536.0, 1024.0, 512.0], dtype=np.float32)[None, :], 1, 0),
            (128, 32),
        ).astype(np.float32),
    }
    _CACHE["consts"] = consts
    return consts


def _make_in_maps(descriptors0, descriptors1):
    consts = _make_consts()
    in_maps = []
    for c in range(B):
        a = np.ascontiguousarray(
            (descriptors0[c] * SCALE).reshape(2, 128, N).transpose(1, 0, 2)
        ).astype(ml_dtypes.float8_e4m3)
        bb = np.ascontiguousarray(
            (descriptors1[c] * SCALE).reshape(2, 128, M).transpose(1, 0, 2)
        ).astype(ml_dtypes.float8_e4m3)
        in_maps.append({"d0": a, "d1": bb, **consts})
    return in_maps


def kernel(descriptors0: np.ndarray, descriptors1: np.ndarray):
    from concourse.bass_utils import run_bass_kernel_spmd

    nc = _get_program()
    in_maps = _make_in_maps(descriptors0, descriptors1)
    res = run_bass_kernel_spmd(nc, in_maps, core_ids=list(range(B)))
    matches = np.stack([np.asarray(res.results[c]["matches"]) for c in range(B)])
    scores = np.stack([np.asarray(res.results[c]["scores"]) for c in range(B)])
    return matches.astype(np.int32), scores.astype(np.float32)


# revision 7
# speedup vs baseline: 1.2075x; 1.0305x over previous
"""Trainium2 Bass kernel for mutual-nearest-neighbor matching (Lowe ratio test).

Batch b=8 sharded 1 element per NeuronCore. Per core:
  sim = d0^T @ d1  [4096, 4096] via fp8-e4m3 DoubleRow matmuls (K=256 in one
  pass, descriptors host-scaled by 16 so sims live in a x256 fp32 domain).

Direction 0 (rows): per 128-row tile, ACT evicts both PSUM halves to bf16 X
  [128, 4096]; DVE folds X -> F1 -> F2 -> F3 (pure tensor_max, full bf16
  precision, no bit embedding), Max8 + FindIndex8 on F3 [512] give
  (v1, v2, F3-slot j*).  The winning column is recovered exactly by one
  gpsimd gather of the 8 comb candidates X[j* + 512k] and a batched
  arithmetic decode in the epilogue.  v2 equals the true second max unless
  the top-2 co-locate in one 8-column comb (harmless for the ratio test).

Direction 1 (columns): the mutual check only needs, per column c, the
  bf16 column max V1[c] and its ratio mask -- no argmax index: row r is the
  column argmax  iff  v1_dir0[r] == V1[c] (bf16 maxes of the same bit-exact
  bf16 sims).  So dir-1 tiles are matmul + fold + Max8 only; a fraction fold
  the second PSUM half directly (tensor_max(SBUF bf16, PSUM fp32)) which
  skips one ACT eviction per tile to balance engine load.  V1 (masked, with
  failed columns set to an impossible value) is bounced through DRAM,
  partition-broadcast, and gathered at m0 for the mutual test.

Engines: PE fp8 matmuls; ACT PSUM evictions; DVE folds/max8/find8;
  Pool (gpsimd) dir-1 F2/F3 folds, candidate gathers and extractions.
"""

import sys

if "/opt/trn_rl_repo" not in sys.path:
    sys.path.insert(0, "/opt/trn_rl_repo")

import numpy as np
import ml_dtypes

B, D, N, M = 8, 256, 4096, 4096
NT = N // 128            # 32 row tiles per direction
HALF = M // 2            # 2048 columns per PSUM half-tile
SCALE = 16.0             # host descriptor scale; sims carry SCALE^2 = 256
RATIO2 = 0.8 * 0.8
THRESH = (1.0 - RATIO2) * SCALE * SCALE   # 0.36 * 256 = 92.16
IMPOSSIBLE = 2.1 * SCALE * SCALE          # > any sim*256
# dir-1 tiles with (t % 3 != 0) evict both halves via ACT;
# the rest fold the second half straight from PSUM on DVE.

_CACHE: dict = {}


def _build_program():
    import concourse.mybir as mybir
    import concourse.tile as tile
    from concourse import bacc

    dt = mybir.dt
    Alu = mybir.AluOpType
    DR = mybir.MatmulPerfMode.DoubleRow

    nc = bacc.Bacc("TRN2", target_bir_lowering=False, debug=False)

    d0_dram = nc.dram_tensor("d0", [128, 2, N], dt.float8e4, kind="ExternalInput")
    d1_dram = nc.dram_tensor("d1", [128, 2, M], dt.float8e4, kind="ExternalInput")
    matches_dram = nc.dram_tensor("matches", [N], dt.int32, kind="ExternalOutput")
    scores_dram = nc.dram_tensor("scores", [N], dt.float32, kind="ExternalOutput")
    v1_bounce = nc.dram_tensor("v1_bounce", [M], dt.float32)  # internal
    c_off8_dram = nc.dram_tensor("c_off8", [128, 8], dt.uint16, kind="ExternalInput")
    c_diag128_dram = nc.dram_tensor("c_diag128", [128, 128], dt.bfloat16, kind="ExternalInput")
    c_diagf512_dram = nc.dram_tensor("c_diagf512", [128, 512], dt.float32, kind="ExternalInput")
    c_prio_dram = nc.dram_tensor("c_prio", [128, 128], dt.float32, kind="ExternalInput")

    with tile.TileContext(nc) as tc:
        with (
            tc.tile_pool(name="w", bufs=1) as wpool,
            tc.tile_pool(name="acc", bufs=1) as apool,
            tc.tile_pool(name="x", bufs=4) as xpool,
            tc.tile_pool(name="f", bufs=4) as fpool,
            tc.tile_pool(name="g", bufs=4) as gpool,
            tc.tile_pool(name="psum", bufs=2, space="PSUM") as ppool,
        ):
            # ---- load descriptors (fp8, k = subtile*128 + partition) ----
            d0_sb = wpool.tile([128, 2, N], dt.float8e4, name="d0")
            d1_sb = wpool.tile([128, 2, M], dt.float8e4, name="d1")
            nc.sync.dma_start(d1_sb[:, :, : M // 2], d1_dram[:, :, : M // 2])
            nc.sync.dma_start(d0_sb[:, :, : N // 2], d0_dram[:, :, : N // 2])
            nc.sync.dma_start(d1_sb[:, :, M // 2 :], d1_dram[:, :, M // 2 :])
            nc.sync.dma_start(d0_sb[:, :, N // 2 :], d0_dram[:, :, N // 2 :])

            # ---- constants ----
            c_off8 = wpool.tile([128, 8], dt.uint16, name="c_off8")
            nc.sync.dma_start(c_off8[:], c_off8_dram[:])
            c_diag128 = wpool.tile([128, 128], dt.bfloat16, name="c_diag128")
            nc.sync.dma_start(c_diag128[:], c_diag128_dram[:])
            c_diagf512 = wpool.tile([128, 512], dt.float32, name="c_diagf512")
            nc.sync.dma_start(c_diagf512[:], c_diagf512_dram[:])
            c_prio = wpool.tile([128, 128], dt.float32, name="c_prio")
            nc.sync.dma_start(c_prio[:], c_prio_dram[:])

            # ---- accumulators ----
            t8a0 = apool.tile([128, NT * 8], dt.bfloat16, name="t8a0")
            t8a1 = apool.tile([128, NT * 8], dt.bfloat16, name="t8a1")
            jacc = apool.tile([128, NT], dt.float32, name="jacc")
            gacc = apool.tile([128, NT * 8], dt.float32, name="gacc")
            V1R = apool.tile([128, M], dt.float32, name="V1R")
            v1_1 = apool.tile([128, NT], dt.float32, name="v1_1")
            v2_1 = apool.tile([128, NT], dt.float32, name="v2_1")
            r1 = apool.tile([128, NT], dt.float32, name="r1e")
            mask1 = apool.tile([128, NT], dt.uint8, name="mask1")
            V1M = apool.tile([128, NT], dt.float32, name="V1M")

            def mm_tile(P, lhs, rhs, t, h):
                for bk in range(4):
                    nc.tensor.matmul(
                        P[:, 512 * bk : 512 * (bk + 1)],
                        lhs[:, :, 128 * t : 128 * (t + 1)],
                        rhs[:, :, HALF * h + 512 * bk : HALF * h + 512 * (bk + 1)],
                        start=True,
                        stop=True,
                        perf_mode=DR,
                    )

            def dir1_tile(t):
                evict = (t % 3) != 0
                P0 = ppool.tile([128, HALF], dt.float32, name=f"q0_{t}", tag="P")
                mm_tile(P0, d1_sb, d0_sb, t, 0)
                P1 = ppool.tile([128, HALF], dt.float32, name=f"q1_{t}", tag="P")
                mm_tile(P1, d1_sb, d0_sb, t, 1)
                F1q = fpool.tile([128, HALF], dt.bfloat16, name=f"f1q_{t}", tag="F1")
                Xq = xpool.tile([128, M], dt.bfloat16, name=f"xq_{t}", tag="X")
                if evict:
                    nc.scalar.copy(Xq[:, :HALF], P0[:])
                    nc.scalar.copy(Xq[:, HALF:], P1[:])
                    nc.vector.tensor_max(F1q[:], Xq[:, :HALF], Xq[:, HALF:])
                else:
                    nc.scalar.copy(Xq[:, :HALF], P0[:])
                    nc.vector.tensor_max(F1q[:], Xq[:, :HALF], P1[:])
                F2q = fpool.tile([128, 1024], dt.bfloat16, name=f"f2q_{t}", tag="F2")
                nc.vector.tensor_max(F2q[:], F1q[:, :1024], F1q[:, 1024:])
                F3q = fpool.tile([128, 512], dt.bfloat16, name=f"f3q_{t}", tag="F3")
                nc.vector.tensor_max(F3q[:], F2q[:, :512], F2q[:, 512:])
                nc.vector.max(t8a1[:, 8 * t : 8 * t + 8], F3q[:])

            def dir1_epilogue():
                nc.vector.tensor_copy(v1_1[:], t8a1[:].rearrange("p (t e) -> p t e", e=8)[:, :, 0])
                nc.vector.tensor_copy(v2_1[:], t8a1[:].rearrange("p (t e) -> p t e", e=8)[:, :, 1])
                nc.vector.scalar_tensor_tensor(
                    r1[:], v2_1[:], -RATIO2, v1_1[:], op0=Alu.mult, op1=Alu.add
                )
                nc.vector.tensor_scalar(mask1[:], r1[:], THRESH, None, op0=Alu.is_ge)
                nc.vector.memset(V1M[:], IMPOSSIBLE)
                nc.vector.copy_predicated(V1M[:], mask1[:], v1_1[:])
                nc.sync.dma_start(v1_bounce[:].rearrange("(t r) -> r t", r=128), V1M[:])
                nc.sync.dma_start(
                    V1R[:], v1_bounce[:][None, :].partition_broadcast(128)
                )

            u = 0
            for t in range(NT):
                if u < NT:
                    dir1_tile(u)
                    u += 1
                if t % 8 == 3 and u < NT:
                    dir1_tile(u)
                    u += 1
                if u == NT:
                    dir1_epilogue()
                    u += 1

                # ================= direction 0 (rows), tile t ===============
                R0 = ppool.tile([128, HALF], dt.float32, name=f"r0_{t}", tag="P")
                mm_tile(R0, d0_sb, d1_sb, t, 0)
                R1 = ppool.tile([128, HALF], dt.float32, name=f"r1_{t}", tag="P")
                mm_tile(R1, d0_sb, d1_sb, t, 1)
                X = xpool.tile([128, M], dt.bfloat16, name=f"x_{t}", tag="X")
                nc.scalar.copy(X[:, :HALF], R0[:])
                nc.scalar.copy(X[:, HALF:], R1[:])
                F1 = fpool.tile([128, HALF], dt.bfloat16, name=f"f1_{t}", tag="F1")
                nc.vector.tensor_max(F1[:], X[:, :HALF], X[:, HALF:])
                F2 = fpool.tile([128, 1024], dt.bfloat16, name=f"f2_{t}", tag="F2")
                nc.vector.tensor_max(F2[:], F1[:, :1024], F1[:, 1024:])
                F3 = fpool.tile([128, 512], dt.bfloat16, name=f"f3_{t}", tag="F3")
                nc.vector.tensor_max(F3[:], F2[:, :512], F2[:, 512:])
                t8s = t8a0[:, 8 * t : 8 * t + 8]
                nc.vector.max(t8s, F3[:])
                pi = gpool.tile([128, 8], dt.uint16, name=f"pi_{t}", tag="pi")
                nc.vector.max_index(pi[:], t8s, F3[:])
                nc.vector.tensor_copy(jacc[:, t : t + 1], pi[:, 0:1])
                idx8 = gpool.tile([128, 8], dt.uint16, name=f"ix_{t}", tag="ix")
                nc.vector.tensor_scalar(
                    idx8[:], c_off8[:], jacc[:, t : t + 1], None, op0=Alu.add
                )
                gth = gpool.tile([128, 128], dt.bfloat16, name=f"gt_{t}", tag="gt")
                nc.gpsimd.indirect_copy(gth[:], X[:], idx8[:], True)
                gd = gpool.tile([128, 128], dt.float32, name=f"gd_{t}", tag="gd")
                nc.vector.tensor_mul(gd[:], gth[:], c_diag128[:])
                nc.vector.tensor_reduce(
                    gacc[:, 8 * t : 8 * t + 8],
                    gd[:].rearrange("p (j u) -> p j u", u=16),
                    axis=mybir.AxisListType.X,
                    op=Alu.add,
                )

            # ============== dir-0 epilogue: decode + mask + mutual ==========
            v1_0 = apool.tile([128, NT], dt.float32, name="v1_0")
            nc.vector.tensor_copy(v1_0[:], t8a0[:].rearrange("p (t e) -> p t e", e=8)[:, :, 0])
            v2_0 = apool.tile([128, NT], dt.float32, name="v2_0")
            nc.vector.tensor_copy(v2_0[:], t8a0[:].rearrange("p (t e) -> p t e", e=8)[:, :, 1])

            # comb decode: gacc[:, 8t+k] = X[j*+512k] (k<4 left, k>=4 right)
            gv = gacc[:].rearrange("p (t k) -> p t k", k=8)
            XLc = gv[:, :, 0:4]
            XRc = gv[:, :, 4:8]
            F1c = apool.tile([128, NT * 4], dt.float32, name="F1c")
            F1cv = F1c[:].rearrange("p (t k) -> p t k", k=4)
            nc.vector.tensor_tensor(F1cv, XLc, XRc, op=Alu.max)
            # v1 broadcast x4
            v1x4 = apool.tile([128, NT * 4], dt.float32, name="v1x4")
            v1x4v = v1x4[:].rearrange("p (t k) -> p t k", k=4)
            for k in range(4):
                nc.vector.tensor_copy(
                    v1x4v[:, :, k : k + 1],
                    v1_0[:].rearrange("p (t o) -> p t o", o=1),
                )
            eqk = apool.tile([128, NT * 4], dt.uint8, name="eqk")
            nc.vector.tensor_tensor(eqk[:], F1c[:], v1x4[:], op=Alu.is_equal)
            sck = apool.tile([128, NT * 4], dt.float32, name="sck")
            nc.vector.tensor_mul(sck[:], eqk[:], c_prio[:])
            mo = apool.tile([128, NT], dt.float32, name="mo")
            nc.vector.tensor_reduce(
                mo[:], sck[:].rearrange("p (t k) -> p t k", k=4),
                axis=mybir.AxisListType.X, op=Alu.max,
            )
            # l_off = 2048 - mo ; onehot = (sck == mo)
            l_off = apool.tile([128, NT], dt.float32, name="l_off")
            nc.vector.tensor_scalar(l_off[:], mo[:], -1.0, 2048.0, op0=Alu.mult, op1=Alu.add)
            mox4 = apool.tile([128, NT * 4], dt.float32, name="mox4")
            mox4v = mox4[:].rearrange("p (t k) -> p t k", k=4)
            for k in range(4):
                nc.vector.tensor_copy(
                    mox4v[:, :, k : k + 1],
                    mo[:].rearrange("p (t o) -> p t o", o=1),
                )
            onehot = apool.tile([128, NT * 4], dt.uint8, name="onehot")
            nc.vector.tensor_tensor(onehot[:], sck[:], mox4[:], op=Alu.is_equal)
            XLs = apool.tile([128, NT * 4], dt.float32, name="XLs")
            nc.vector.tensor_tensor(XLs[:], onehot[:], XLc, op=Alu.mult)
            XLsel = apool.tile([128, NT], dt.float32, name="XLsel")
            nc.vector.tensor_reduce(
                XLsel[:], XLs[:].rearrange("p (t k) -> p t k", k=4),
                axis=mybir.AxisListType.X, op=Alu.add,
            )
            bitR = apool.tile([128, NT], dt.uint8, name="bitR")
            nc.vector.tensor_tensor(bitR[:], XLsel[:], v1_0[:], op=Alu.is_lt)
            # col = j* + l_off + 2048*bitR
            colf = apool.tile([128, NT], dt.float32, name="colf")
            nc.vector.tensor_tensor(colf[:], jacc[:], l_off[:], op=Alu.add)
            nc.vector.scalar_tensor_tensor(
                colf[:], bitR[:], 2048.0, colf[:], op0=Alu.mult, op1=Alu.add
            )

            # ratio mask + scores
            r0e = apool.tile([128, NT], dt.float32, name="r0e")
            nc.vector.scalar_tensor_tensor(
                r0e[:], v2_0[:], -RATIO2, v1_0[:], op0=Alu.mult, op1=Alu.add
            )
            mask0 = apool.tile([128, NT], dt.uint8, name="mask0")
            nc.vector.tensor_scalar(mask0[:], r0e[:], THRESH, None, op0=Alu.is_ge)
            sc = apool.tile([128, NT], dt.float32, name="sc")
            nc.vector.tensor_scalar(
                sc[:], v1_0[:], 0.5 / (SCALE * SCALE), 0.5, op0=Alu.mult, op1=Alu.add
            )
            scores0 = apool.tile([128, NT], dt.float32, name="scores0")
            nc.vector.tensor_mul(scores0[:], sc[:], mask0[:])

            m0 = apool.tile([128, NT], dt.float32, name="m0")
            nc.vector.memset(m0[:], -1.0)
            nc.vector.copy_predicated(m0[:], mask0[:], colf[:])

            # mutual: gather V1R at columns m0 and compare with v1_0
            safe = apool.tile([128, NT], dt.float32, name="safe")
            nc.vector.tensor_scalar_max(safe[:], m0[:], 0.0)
            safe16 = apool.tile([128, NT], dt.uint16, name="safe16")
            nc.vector.tensor_copy(safe16[:], safe[:])
            gvr = apool.tile([128, 16 * NT], dt.float32, name="gvr")
            nc.gpsimd.indirect_copy(gvr[:], V1R[:], safe16[:], True)
            gvd = apool.tile([128, 16 * NT], dt.float32, name="gvd")
            nc.vector.tensor_mul(gvd[:], gvr[:], c_diagf512[:])
            V1at = apool.tile([128, NT], dt.float32, name="V1at")
            nc.vector.tensor_reduce(
                V1at[:], gvd[:].rearrange("p (j u) -> p j u", u=16),
                axis=mybir.AxisListType.X, op=Alu.add,
            )
            okb = apool.tile([128, NT], dt.uint8, name="okb")
            nc.vector.tensor_tensor(okb[:], V1at[:], v1_0[:], op=Alu.is_equal)
            ok = apool.tile([128, NT], dt.uint8, name="ok")
            nc.vector.tensor_mul(ok[:], okb[:], mask0[:])

            mfin = apool.tile([128, NT], dt.float32, name="mfin")
            nc.vector.memset(mfin[:], -1.0)
            nc.vector.copy_predicated(mfin[:], ok[:], m0[:])
            mi32 = apool.tile([128, NT], dt.int32, name="mi32")
            nc.vector.tensor_copy(mi32[:], mfin[:])

            nc.sync.dma_start(matches_dram[:].rearrange("(t r) -> r t", r=128), mi32[:])
            nc.sync.dma_start(scores_dram[:].rearrange("(t r) -> r t", r=128), scores0[:])

    nc.compile()
    return nc


def _get_program():
    if "nc" not in _CACHE:
        _CACHE["nc"] = _build_program()
    return _CACHE["nc"]


def _make_consts():
    if "consts" in _CACHE:
        return _CACHE["consts"]
    p = np.arange(128)
    diag16 = (np.arange(16)[None, :] == (p % 16)[:, None])  # [128, 16]
    off8 = np.array([0, 512, 1024, 1536, 2048, 2560, 3072, 3584], dtype=np.uint16)
    consts = {
        "c_off8": np.tile(off8[None, :], (128, 1)).astype(np.uint16),
        "c_diag128": np.tile(diag16, (1, 8)).astype(ml_dtypes.bfloat16),
        "c_diagf512": np.tile(diag16, (1, 32)).astype(np.float32),
        "c_prio": np.tile(
            np.repeat(np.array([2048.0, 1536.0, 1024.0, 512.0], dtype=np.float32)[None, :], 1, 0),
            (128, 32),
        ).astype(np.float32),
    }
    _CACHE["consts"] = consts
    return consts


def _make_in_maps(descriptors0, descriptors1):
    consts = _make_consts()
    in_maps = []
    for c in range(B):
        a = np.ascontiguousarray(
            (descriptors0[c] * SCALE).reshape(2, 128, N).transpose(1, 0, 2)
        ).astype(ml_dtypes.float8_e4m3)
        bb = np.ascontiguousarray(
            (descriptors1[c] * SCALE).reshape(2, 128, M).transpose(1, 0, 2)
        ).astype(ml_dtypes.float8_e4m3)
        in_maps.append({"d0": a, "d1": bb, **consts})
    return in_maps


def kernel(descriptors0: np.ndarray, descriptors1: np.ndarray):
    from concourse.bass_utils import run_bass_kernel_spmd

    nc = _get_program()
    in_maps = _make_in_maps(descriptors0, descriptors1)
    res = run_bass_kernel_spmd(nc, in_maps, core_ids=list(range(B)))
    matches = np.stack([np.asarray(res.results[c]["matches"]) for c in range(B)])
    scores = np.stack([np.asarray(res.results[c]["scores"]) for c in range(B)])
    return matches.astype(np.int32), scores.astype(np.float32)
